# revision 20
# baseline (speedup 1.0000x reference)
"""Multi-head attention forward on 8 Trainium2 NeuronCores.

Sharding: batch (2) x head-groups (4 heads each) -> 8 cores, Megatron-style.
Each core computes q/k/v projections for its 256-dim head slice, attention
for its 4 heads, and a partial output projection; the host sums the 4
partials per batch element and adds the output bias.

Device-side layout choices (all picked to avoid fp32 transposes on chip):
 - host passes x^T (embed-major) activations, so the projection matmuls
   contract embed on partitions directly
 - q and k are produced head-transposed [d, s]; the scores matmul
   (lhsT=k^T chunk, rhs=q^T) then emits scores^T [k_seq, q_seq] whose
   partition dim is k_seq -- exactly what the ctx matmul needs to contract
 - softmax skips max-subtraction (scores ~ N(0,1), |s| < ~6 => exp is safe
   in fp32); the denominator Z rides along as a fused ones-column of v in
   the ctx matmul (lhsT = [v_h | 1], M=65)
 - normalization by 1/Z commutes past nothing (per-head Z), so ctx^T is
   scaled via gpsimd partition_broadcast of the reciprocal row
"""

import numpy as np
import ml_dtypes

import concourse.bass as bass
import concourse.tile as tile
from concourse import bacc, mybir
from concourse.bass_utils import run_bass_kernel_spmd

F32 = mybir.dt.float32
F32R = mybir.dt.float32r
BF16 = mybir.dt.bfloat16

B = 2
S = 2048
E = 1024
H = 16
D = 64
HPC = 4          # heads per core
EC = HPC * D     # 256: embed slice per core
NCORES = 8
KO = E // 128    # 8 contraction chunks for the projections


def build_mha(tc: tile.TileContext, S_=S, reps=1):
    nc = tc.nc
    SI = S_ // 512       # 512-wide seq chunks
    JC = S_ // 128       # 128-wide key chunks
    NH = S_ // 1024 if S_ >= 1024 else 1   # i-halves
    IW = min(S_, 1024)   # i-block width
    IIW = IW // 512      # 512-wide slices per i-block

    xq = nc.dram_tensor("xq", [E, S_], F32R, kind="ExternalInput").ap()
    xk = nc.dram_tensor("xk", [E, S_], F32R, kind="ExternalInput").ap()
    xv = nc.dram_tensor("xv", [E, S_], BF16, kind="ExternalInput").ap()
    wq = nc.dram_tensor("wq", [E, EC], F32R, kind="ExternalInput").ap()
    wk = nc.dram_tensor("wk", [E, EC], F32R, kind="ExternalInput").ap()
    wv = nc.dram_tensor("wv", [E, EC], BF16, kind="ExternalInput").ap()
    wo = nc.dram_tensor("wo", [EC, E], F32R, kind="ExternalInput").ap()
    bq = nc.dram_tensor("bq", [EC], F32, kind="ExternalInput").ap()
    bk = nc.dram_tensor("bk", [EC], F32, kind="ExternalInput").ap()
    bv = nc.dram_tensor("bv", [EC], F32, kind="ExternalInput").ap()
    out = nc.dram_tensor("out", [S_, E], F32, kind="ExternalOutput").ap()

    xq3 = xq.rearrange("(ko p) s -> p ko s", p=128)
    xk3 = xk.rearrange("(ko p) s -> p ko s", p=128)
    xv3 = xv.rearrange("(ko p) s -> p ko s", p=128)

    for _rep in range(reps):
      with (
        tc.tile_pool(name="wpool", bufs=1) as wpool,
        tc.tile_pool(name="persist", bufs=1) as persist,
        tc.tile_pool(name="xin", bufs=3) as xin,
        tc.tile_pool(name="xinv", bufs=2) as xinv,
        tc.tile_pool(name="expp", bufs=5) as expp,
        tc.tile_pool(name="csbp", bufs=2) as csbp,
        tc.tile_pool(name="rzp", bufs=2) as rzp,
        tc.tile_pool(name="rzbp", bufs=2) as rzbp,
        tc.tile_pool(name="outp", bufs=3) as outp,
        tc.tile_pool(name="psA", bufs=2, space="PSUM") as psA,
        tc.tile_pool(name="psS", bufs=2, space="PSUM") as psS,
        tc.tile_pool(name="psC", bufs=2, space="PSUM") as psC,
    ):
        # ---- weights / biases / persistent tiles ----
        wq_sb = wpool.tile([128, KO, EC], F32R)
        wk_sb = wpool.tile([128, KO, EC], F32R)
        wv_sb = wpool.tile([128, KO, EC], BF16)
        wo_sb = wpool.tile([128, 2, E], F32R)
        bq_sb = wpool.tile([128, 2], F32)
        bk_sb = wpool.tile([128, 2], F32)
        bv_row = wpool.tile([1, EC], F32)
        bv_bc = wpool.tile([128, EC], F32)

        qT = persist.tile([128, 2, S_], F32R)   # [d(2 heads), head-pair, s]
        kT = persist.tile([128, 2, S_], F32R)
        va = persist.tile([128, JC, HPC * 65], F32R)  # [s%128, s//128, h*(64+1)]
        ctxn = persist.tile([128, 2, S_], F32R)       # normalized ctx^T

        va4 = va[:].bitcast(F32).rearrange("p j (h t) -> p j h t", t=65)
        nc.vector.memset(va4[:, :, :, 64], 1.0)

        # ---- projections ----
        # Emission order is the DMA-queue order; attention for i-half 0
        # unlocks after wq + xq(si 0,1) + wk + xk(all) + wv + xv(si 0), so
        # stream those first and let the rest arrive during attention.
        def q_proj(si):
            sl = bass.ts(si, 512)
            xq_t = xin.tile([128, KO, 512], F32R, tag="xin")
            nc.sync.dma_start(xq_t[:], xq3[:, :, sl])
            for c in range(2):
                pq = psA.tile([128, 512], F32, tag="ps_a")
                for ko in range(KO):
                    nc.tensor.matmul(pq[:], wq_sb[:, ko, bass.ts(c, 128)],
                                     xq_t[:, ko, :],
                                     start=(ko == 0), stop=(ko == KO - 1))
                nc.vector.tensor_scalar_add(qT[:, c, sl], pq[:], bq_sb[:, c:c + 1])

        def k_proj(si):
            sl = bass.ts(si, 512)
            xk_t = xin.tile([128, KO, 512], F32R, tag="xin")
            nc.sync.dma_start(xk_t[:], xk3[:, :, sl])
            for c in range(2):
                pk = psA.tile([128, 512], F32, tag="ps_a")
                for ko in range(KO):
                    nc.tensor.matmul(pk[:], wk_sb[:, ko, bass.ts(c, 128)],
                                     xk_t[:, ko, :],
                                     start=(ko == 0), stop=(ko == KO - 1))
                nc.vector.tensor_scalar_add(kT[:, c, sl], pk[:], bk_sb[:, c:c + 1])

        def v_proj(si):
            sl = bass.ts(si, 512)
            xv_t = xinv.tile([128, KO, 512], BF16, tag="xin_v")
            nc.sync.dma_start(xv_t[:], xv3[:, :, sl])
            for sj in range(4):
                jc = si * 4 + sj
                pv = psA.tile([128, 512], F32, tag="ps_a")
                for ko in range(KO):
                    nc.tensor.matmul(pv[:, 0:EC],
                                     xv_t[:, ko, bass.ts(sj, 128)],
                                     wv_sb[:, ko, :],
                                     start=(ko == 0), stop=(ko == KO - 1))
                for h in range(HPC):
                    nc.vector.tensor_add(va[:, jc, h * 65:h * 65 + 64],
                                         pv[:, bass.ts(h, 64)],
                                         bv_bc[:, bass.ts(h, 64)])

        nc.sync.dma_start(wq_sb[:], wq.rearrange("(ko p) m -> p ko m", p=128))
        nc.sync.dma_start(bq_sb[:], bq.rearrange("(c p) -> p c", p=128))
        n_q_early = min(1, SI)
        for si in range(n_q_early):
            q_proj(si)
        nc.sync.dma_start(wk_sb[:], wk.rearrange("(ko p) m -> p ko m", p=128))
        nc.sync.dma_start(bk_sb[:], bk.rearrange("(c p) -> p c", p=128))
        for si in range(SI):
            k_proj(si)
        nc.sync.dma_start(wv_sb[:], wv.rearrange("(ko p) m -> p ko m", p=128))
        nc.sync.dma_start(bv_row[:], bv[None, :])
        nc.gpsimd.partition_broadcast(bv_bc[:], bv_row[:])
        for si in range(SI):
            v_proj(si)
        nc.sync.dma_start(wo_sb[:], wo.rearrange("(kf p) e -> p kf e", p=128))
        q_late = list(range(n_q_early, SI))

        # ---- attention + output projection ----
        # Head pairs (0,1) and (2,3) are processed together per i-quarter:
        # the pair's scores matmuls are K=64 on PE row-tiles (0,*) and
        # (64,*), emitted back-to-back so the hardware can overlap them,
        # and share one [128, 1024] psum tile -> one exp instruction.
        NQ = S_ // 512
        for qt in range(NQ):
            for pair in range(2):
                if q_late:
                    q_proj(q_late.pop(0))
                isl = bass.ts(qt, 512)
                C2 = [psC.tile([65, 512], F32, tag="ps_c", name=f"C{hh}")
                      for hh in range(2)]
                for jc in range(JC):
                    S_t = psS.tile([128, 1024], F32, tag="ps_s")
                    for hh in range(2):
                        nc.tensor.matmul(S_t[:, bass.ts(hh, 512)],
                                         kT[hh * 64:hh * 64 + 64, pair,
                                            bass.ts(jc, 128)],
                                         qT[hh * 64:hh * 64 + 64, pair, isl],
                                         start=True, stop=True)
                    eT = expp.tile([128, 1024], F32R, tag="expp")
                    nc.scalar.activation(eT[:], S_t[:],
                                         mybir.ActivationFunctionType.Exp)
                    for hh in range(2):
                        h = 2 * pair + hh
                        nc.tensor.matmul(C2[hh][:],
                                         va[:, jc, h * 65:h * 65 + 65],
                                         eT[:, bass.ts(hh, 512)],
                                         start=(jc == 0), stop=(jc == JC - 1))
                # normalize: ctxn = C[0:64] / C[64]
                for hh in range(2):
                    csb = csbp.tile([65, 512], F32, tag="csb")
                    nc.vector.tensor_copy(csb[:], C2[hh][:])
                    rz = rzp.tile([1, 512], F32, tag="rz")
                    nc.vector.reciprocal(rz[0:1, :], csb[64:65, :])
                    rzb = rzbp.tile([64, 512], F32, tag="rzb")
                    nc.gpsimd.partition_broadcast(rzb[:], rz[:])
                    nc.vector.tensor_tensor(ctxn[hh * 64:hh * 64 + 64, pair,
                                                 isl],
                                            csb[0:64, :], rzb[:],
                                            mybir.AluOpType.mult)
            # output projection for this i-quarter; the kf=0 half is a
            # separate GEMM combined by a DVE add so it can run as soon as
            # heads 0,1 are normalized (psum doesn't straddle the kf pair)
            for sc in range(4):
                s0 = qt * 512 + sc * 128
                for eo in range(2):
                    p0t = psA.tile([128, 512], F32, tag="ps_a")
                    nc.tensor.matmul(p0t[:], ctxn[:, 0, bass.ds(s0, 128)],
                                     wo_sb[:, 0, bass.ts(eo, 512)],
                                     start=True, stop=True)
                    ot = outp.tile([128, 512], F32, tag="ot")
                    nc.vector.tensor_copy(ot[:], p0t[:])
                    p1t = psA.tile([128, 512], F32, tag="ps_a")
                    nc.tensor.matmul(p1t[:], ctxn[:, 1, bass.ds(s0, 128)],
                                     wo_sb[:, 1, bass.ts(eo, 512)],
                                     start=True, stop=True)
                    nc.vector.tensor_add(ot[:], ot[:], p1t[:])
                    nc.sync.dma_start(out[bass.ds(s0, 128), bass.ts(eo, 512)],
                                      ot[:])


_CACHED = {}


def _get_nc(S_=S, reps=1):
    key = (S_, reps)
    if key not in _CACHED:
        nc = bacc.Bacc("TRN2", target_bir_lowering=False, debug=False)
        with tile.TileContext(nc) as tc:
            build_mha(tc, S_, reps)
        nc.compile()
        _CACHED[key] = nc
    return _CACHED[key]


def shard_inputs(query, key, value, Wq, bq, Wk, bk, Wv, bv, Wo, bo):
    """Build the 8 per-core input maps (numpy, fp32)."""
    scale = np.float32(1.0 / np.sqrt(D))
    in_maps = []
    for core in range(NCORES):
        b = core // HPC
        g = core % HPC
        hs = slice(g * EC, (g + 1) * EC)
        in_maps.append({
            "xq": np.ascontiguousarray(query[b].T, np.float32),
            "xk": np.ascontiguousarray(key[b].T, np.float32),
            "xv": np.ascontiguousarray(value[b].T).astype(ml_dtypes.bfloat16),
            "wq": np.ascontiguousarray(Wq[hs, :].T, np.float32),
            "wk": np.ascontiguousarray(Wk[hs, :].T * scale, np.float32),
            "wv": np.ascontiguousarray(Wv[hs, :].T).astype(ml_dtypes.bfloat16),
            "wo": np.ascontiguousarray(Wo[:, hs].T, np.float32),
            "bq": np.ascontiguousarray(bq[hs], np.float32),
            "bk": np.ascontiguousarray(bk[hs] * scale, np.float32),
            "bv": np.ascontiguousarray(bv[hs], np.float32),
        })
    return in_maps


def combine_outputs(results, bo):
    out = np.zeros((B, S, E), np.float32)
    for core in range(NCORES):
        out[core // HPC] += results[core]["out"]
    out += np.asarray(bo, np.float32)[None, None, :]
    return out


def kernel(query, key, value, Wq, bq, Wk, bk, Wv, bv, Wo, bo):
    nc = _get_nc()
    in_maps = shard_inputs(query, key, value, Wq, bq, Wk, bk, Wv, bv, Wo, bo)
    res = run_bass_kernel_spmd(nc, in_maps, list(range(NCORES)))
    return combine_outputs(res.results, bo)


# revision 24
# speedup vs baseline: 1.3224x; 1.3224x over previous
"""Multi-head attention forward on 8 Trainium2 NeuronCores.

Sharding: batch (2) x head-groups (4 heads each) -> 8 cores, Megatron-style.
Each core computes q/k/v projections for its 256-dim head slice, attention
for its 4 heads, and a partial output projection; the host sums the 4
partials per batch element and adds the output bias.

Device-side layout choices (all picked to avoid fp32 transposes on chip):
 - host passes x^T (embed-major) activations, so the projection matmuls
   contract embed on partitions directly
 - q and k are produced head-transposed [d, s]; the scores matmul
   (lhsT=k^T chunk, rhs=q^T) then emits scores^T [k_seq, q_seq] whose
   partition dim is k_seq -- exactly what the ctx matmul needs to contract
 - softmax skips max-subtraction (scores ~ N(0,1), |s| < ~6 => exp is safe
   in fp32); the denominator Z rides along as a fused ones-column of v in
   the ctx matmul (lhsT = [v_h | 1], M=65)
 - normalization by 1/Z commutes past nothing (per-head Z), so ctx^T is
   scaled via gpsimd partition_broadcast of the reciprocal row
"""

import numpy as np
import ml_dtypes

import concourse.bass as bass
import concourse.tile as tile
from concourse import bacc, mybir
from concourse.bass_utils import run_bass_kernel_spmd

F32 = mybir.dt.float32
F32R = mybir.dt.float32r
BF16 = mybir.dt.bfloat16

B = 2
S = 2048
E = 1024
H = 16
D = 64
HPC = 4          # heads per core
EC = HPC * D     # 256: embed slice per core
NCORES = 8
KO = E // 128    # 8 contraction chunks for the projections


def build_mha(tc: tile.TileContext, S_=S, reps=1):
    nc = tc.nc
    SI = S_ // 512       # 512-wide seq chunks
    JC = S_ // 128       # 128-wide key chunks
    NH = S_ // 1024 if S_ >= 1024 else 1   # i-halves
    IW = min(S_, 1024)   # i-block width
    IIW = IW // 512      # 512-wide slices per i-block

    xq = nc.dram_tensor("xq", [E, S_], F32R, kind="ExternalInput").ap()
    xk = nc.dram_tensor("xk", [E, S_], F32R, kind="ExternalInput").ap()
    xv = nc.dram_tensor("xv", [E, S_], BF16, kind="ExternalInput").ap()
    wq = nc.dram_tensor("wq", [E, EC], F32R, kind="ExternalInput").ap()
    wk = nc.dram_tensor("wk", [E, EC], F32R, kind="ExternalInput").ap()
    wv = nc.dram_tensor("wv", [E, EC], BF16, kind="ExternalInput").ap()
    wo = nc.dram_tensor("wo", [EC, E], F32R, kind="ExternalInput").ap()
    bq = nc.dram_tensor("bq", [EC], F32, kind="ExternalInput").ap()
    bk = nc.dram_tensor("bk", [EC], F32, kind="ExternalInput").ap()
    bv = nc.dram_tensor("bv", [EC], F32, kind="ExternalInput").ap()
    out = nc.dram_tensor("out", [S_, E], F32, kind="ExternalOutput").ap()

    xq3 = xq.rearrange("(ko p) s -> p ko s", p=128)
    xk3 = xk.rearrange("(ko p) s -> p ko s", p=128)
    xv3 = xv.rearrange("(ko p) s -> p ko s", p=128)

    for _rep in range(reps):
      with (
        tc.tile_pool(name="wpool", bufs=1) as wpool,
        tc.tile_pool(name="persist", bufs=1) as persist,
        tc.tile_pool(name="xin", bufs=3) as xin,
        tc.tile_pool(name="xinv", bufs=2) as xinv,
        tc.tile_pool(name="expp", bufs=5) as expp,
        tc.tile_pool(name="csbp", bufs=2) as csbp,
        tc.tile_pool(name="rzp", bufs=2) as rzp,
        tc.tile_pool(name="rzbp", bufs=2) as rzbp,
        tc.tile_pool(name="outp", bufs=3) as outp,
        tc.tile_pool(name="psA", bufs=2, space="PSUM") as psA,
        tc.tile_pool(name="psS", bufs=2, space="PSUM") as psS,
        tc.tile_pool(name="psC", bufs=2, space="PSUM") as psC,
    ):
        # ---- weights / biases / persistent tiles ----
        wq_sb = wpool.tile([128, KO, EC], F32R)
        wk_sb = wpool.tile([128, KO, EC], F32R)
        wv_sb = wpool.tile([128, KO, EC], BF16)
        wo_sb = wpool.tile([128, 2, E], F32R)
        bq_sb = wpool.tile([128, 2], F32)
        bk_sb = wpool.tile([128, 2], F32)
        bv_row = wpool.tile([1, EC], F32)
        bv_bc = wpool.tile([128, EC], F32)

        qT = persist.tile([128, 2, S_], F32R)   # [d(2 heads), head-pair, s]
        kT = persist.tile([128, 2, S_], F32R)
        va = persist.tile([128, JC, HPC * 65], F32R)  # [s%128, s//128, h*(64+1)]
        ctxn = persist.tile([128, 2, S_], F32R)       # normalized ctx^T

        va4 = va[:].bitcast(F32).rearrange("p j (h t) -> p j h t", t=65)
        nc.vector.memset(va4[:, :, :, 64], 1.0)

        # ---- projections ----
        # Emission order is the DMA-queue order; attention for i-half 0
        # unlocks after wq + xq(si 0,1) + wk + xk(all) + wv + xv(si 0), so
        # stream those first and let the rest arrive during attention.
        def q_proj(si):
            sl = bass.ts(si, 512)
            xq_t = xin.tile([128, KO, 512], F32R, tag="xin")
            nc.sync.dma_start(xq_t[:], xq3[:, :, sl])
            for c in range(2):
                pq = psA.tile([128, 512], F32, tag="ps_a")
                for ko in range(KO):
                    nc.tensor.matmul(pq[:], wq_sb[:, ko, bass.ts(c, 128)],
                                     xq_t[:, ko, :],
                                     start=(ko == 0), stop=(ko == KO - 1))
                nc.vector.tensor_scalar_add(qT[:, c, sl], pq[:], bq_sb[:, c:c + 1])

        def k_proj(si):
            sl = bass.ts(si, 512)
            xk_t = xin.tile([128, KO, 512], F32R, tag="xin")
            nc.sync.dma_start(xk_t[:], xk3[:, :, sl])
            for c in range(2):
                pk = psA.tile([128, 512], F32, tag="ps_a")
                for ko in range(KO):
                    nc.tensor.matmul(pk[:], wk_sb[:, ko, bass.ts(c, 128)],
                                     xk_t[:, ko, :],
                                     start=(ko == 0), stop=(ko == KO - 1))
                nc.vector.tensor_scalar_add(kT[:, c, sl], pk[:], bk_sb[:, c:c + 1])

        def v_proj(si):
            sl = bass.ts(si, 512)
            xv_t = xinv.tile([128, KO, 512], BF16, tag="xin_v")
            nc.sync.dma_start(xv_t[:], xv3[:, :, sl])
            for sj in range(4):
                jc = si * 4 + sj
                pv = psA.tile([128, 512], F32, tag="ps_a")
                for ko in range(KO):
                    nc.tensor.matmul(pv[:, 0:EC],
                                     xv_t[:, ko, bass.ts(sj, 128)],
                                     wv_sb[:, ko, :],
                                     start=(ko == 0), stop=(ko == KO - 1))
                for h in range(HPC):
                    nc.vector.tensor_add(va[:, jc, h * 65:h * 65 + 64],
                                         pv[:, bass.ts(h, 64)],
                                         bv_bc[:, bass.ts(h, 64)])

        nc.sync.dma_start(wq_sb[:], wq.rearrange("(ko p) m -> p ko m", p=128))
        nc.sync.dma_start(bq_sb[:], bq.rearrange("(c p) -> p c", p=128))
        n_q_early = min(1, SI)
        for si in range(n_q_early):
            q_proj(si)
        nc.sync.dma_start(wk_sb[:], wk.rearrange("(ko p) m -> p ko m", p=128))
        nc.sync.dma_start(bk_sb[:], bk.rearrange("(c p) -> p c", p=128))
        for si in range(SI):
            k_proj(si)
        nc.sync.dma_start(wv_sb[:], wv.rearrange("(ko p) m -> p ko m", p=128))
        nc.sync.dma_start(bv_row[:], bv[None, :])
        nc.gpsimd.partition_broadcast(bv_bc[:], bv_row[:])
        for si in range(SI):
            v_proj(si)
        nc.sync.dma_start(wo_sb[:], wo.rearrange("(kf p) e -> p kf e", p=128))
        q_late = list(range(n_q_early, SI))

        # ---- attention + output projection ----
        # Head pairs (0,1) and (2,3) are processed together per i-quarter:
        # the pair's scores matmuls are K=64 on PE row-tiles (0,*) and
        # (64,*), emitted back-to-back so the hardware can overlap them,
        # and share one [128, 1024] psum tile -> one exp instruction.
        NQ = S_ // 512
        for qt in range(NQ):
            for pair in range(2):
                if q_late:
                    q_proj(q_late.pop(0))
                isl = bass.ts(qt, 512)
                C2 = [psC.tile([65, 512], F32, tag="ps_c", name=f"C{hh}")
                      for hh in range(2)]
                for jc in range(JC):
                    S_t = psS.tile([128, 1024], F32, tag="ps_s")
                    for hh in range(2):
                        nc.tensor.matmul(S_t[:, bass.ts(hh, 512)],
                                         kT[hh * 64:hh * 64 + 64, pair,
                                            bass.ts(jc, 128)],
                                         qT[hh * 64:hh * 64 + 64, pair, isl],
                                         start=True, stop=True)
                    eT = expp.tile([128, 1024], F32R, tag="expp")
                    nc.scalar.activation(eT[:], S_t[:],
                                         mybir.ActivationFunctionType.Exp)
                    for hh in range(2):
                        h = 2 * pair + hh
                        nc.tensor.matmul(C2[hh][:],
                                         va[:, jc, h * 65:h * 65 + 65],
                                         eT[:, bass.ts(hh, 512)],
                                         start=(jc == 0), stop=(jc == JC - 1))
                # normalize: ctxn = C[0:64] / C[64]
                for hh in range(2):
                    csb = csbp.tile([65, 512], F32, tag="csb")
                    nc.vector.tensor_copy(csb[:], C2[hh][:])
                    rz = rzp.tile([1, 512], F32, tag="rz")
                    nc.vector.reciprocal(rz[0:1, :], csb[64:65, :])
                    rzb = rzbp.tile([64, 512], F32, tag="rzb")
                    nc.gpsimd.partition_broadcast(rzb[:], rz[:])
                    nc.vector.tensor_tensor(ctxn[hh * 64:hh * 64 + 64, pair,
                                                 isl],
                                            csb[0:64, :], rzb[:],
                                            mybir.AluOpType.mult)
            # output projection for this i-quarter; the kf=0 half is a
            # separate GEMM combined by a DVE add so it can run as soon as
            # heads 0,1 are normalized (psum doesn't straddle the kf pair)
            for sc in range(4):
                s0 = qt * 512 + sc * 128
                for eo in range(2):
                    p0t = psA.tile([128, 512], F32, tag="ps_a")
                    nc.tensor.matmul(p0t[:], ctxn[:, 0, bass.ds(s0, 128)],
                                     wo_sb[:, 0, bass.ts(eo, 512)],
                                     start=True, stop=True)
                    ot = outp.tile([128, 512], F32, tag="ot")
                    nc.vector.tensor_copy(ot[:], p0t[:])
                    p1t = psA.tile([128, 512], F32, tag="ps_a")
                    nc.tensor.matmul(p1t[:], ctxn[:, 1, bass.ds(s0, 128)],
                                     wo_sb[:, 1, bass.ts(eo, 512)],
                                     start=True, stop=True)
                    nc.vector.tensor_add(ot[:], ot[:], p1t[:])
                    nc.sync.dma_start(out[bass.ds(s0, 128), bass.ts(eo, 512)],
                                      ot[:])


_CACHED = {}


def _get_nc(S_=S, reps=1):
    key = (S_, reps)
    if key not in _CACHED:
        nc = bacc.Bacc("TRN2", target_bir_lowering=False, debug=False)
        with tile.TileContext(nc) as tc:
            build_mha(tc, S_, reps)
        nc.compile()
        _CACHED[key] = nc
    return _CACHED[key]


def shard_inputs(query, key, value, Wq, bq, Wk, bk, Wv, bv, Wo, bo):
    """Build the 8 per-core input maps (numpy, fp32)."""
    scale = np.float32(1.0 / np.sqrt(D))
    in_maps = []
    for core in range(NCORES):
        b = core // HPC
        g = core % HPC
        hs = slice(g * EC, (g + 1) * EC)
        in_maps.append({
            "xq": np.ascontiguousarray(query[b].T, np.float32),
            "xk": np.ascontiguousarray(key[b].T, np.float32),
            "xv": np.ascontiguousarray(value[b].T).astype(ml_dtypes.bfloat16),
            "wq": np.ascontiguousarray(Wq[hs, :].T, np.float32),
            "wk": np.ascontiguousarray(Wk[hs, :].T * scale, np.float32),
            "wv": np.ascontiguousarray(Wv[hs, :].T).astype(ml_dtypes.bfloat16),
            "wo": np.ascontiguousarray(Wo[:, hs].T, np.float32),
            "bq": np.ascontiguousarray(bq[hs], np.float32),
            "bk": np.ascontiguousarray(bk[hs] * scale, np.float32),
            "bv": np.ascontiguousarray(bv[hs], np.float32),
        })
    return in_maps


def combine_outputs(results, bo):
    out = np.zeros((B, S, E), np.float32)
    for core in range(NCORES):
        out[core // HPC] += results[core]["out"]
    out += np.asarray(bo, np.float32)[None, None, :]
    return out


def kernel(query, key, value, Wq, bq, Wk, bk, Wv, bv, Wo, bo):
    nc = _get_nc()
    in_maps = shard_inputs(query, key, value, Wq, bq, Wk, bk, Wv, bv, Wo, bo)
    res = run_bass_kernel_spmd(nc, in_maps, list(range(NCORES)))
    return combine_outputs(res.results, bo)
